# revision 2
# baseline (speedup 1.0000x reference)
"""Maxwell viscoelastic model (linear recurrence scan) on 8 Trainium2 NeuronCores.

Math (per trajectory, T timesteps, k = E/eta = 2):
    a_n = 1 - 2*dt_n
    gamma_n = a_n*gamma_{n-1} + 2*dt_n*eps_n,  gamma_0 = 0
    sigma_n = 2.5*eps_n - 2*gamma_n = -4*gamma~_n + 2.5*eps_n   (gamma~ = gamma/2)

Kernel strategy (v3, bf16 I/O):
  Host: deinterleave x[:, :, 0/1] into eps/dt and cast to bf16 (the 2e-2
  rel-err budget makes bf16 transport safe; measured pipeline error ~9e-3).
  Halves HBM traffic: per core 8.4MB in + 4.2MB out -> ~35us DMA floor.

  Per core 512 trajectories = 4 tiles x [128 part, 4096 t], chunks of 1024.
  The tensor_tensor_scan is a fixed-rate DVE op (2.21ns/elem regardless of
  dtype; measured 2263ns/chunk), so DVE runs ONLY the scan:
    ACT   a = 1 - 2*dt -> pa PSUM f32 (scan's d0 via PSUM keeps the DVE off
          the SBUF read port GpSimd needs - measured 2x mutual slowdown if
          both DVE scan operands live in SBUF while Pool runs)
    POOL  m = dt*eps -> SBUF bf16 (plain tensor_tensor; TensorScalarPtr and
          PSUM access are illegal on Pool)
    DVE   scan(pa, m) -> g SBUF bf16 (gamma~), init chained from the
          previous chunk's last column (fp32 state internally)
    PE    sigma = (-4I)^T @ g + (2.5I)^T @ eps -> pq PSUM f32
          (PE accumulates only onto its own group: start=True then False)
    ACT   downcast pq -> sg SBUF bf16
    SP    input tile loads + output stores
"""

import numpy as np
import ml_dtypes

import concourse.bass as bass
import concourse.mybir as mybir
from concourse.bass_utils import run_bass_kernel_spmd

f32 = mybir.dt.float32
bf16 = mybir.dt.bfloat16

N_CORES = 8
P = 128                      # SBUF partitions
T = 4096                     # timesteps
L = 1024                     # chunk length
CH = T // L                  # 4 chunks per tile
BSH = 512                    # trajectories per core
NT = BSH // P                # 4 tiles
NQ = NT * CH                 # 16 chunks


def build_nc() -> bass.Bass:
    nc = bass.Bass()
    dt_d = nc.dram_tensor("dt", [BSH, T], bf16, kind="ExternalInput")
    ep_d = nc.dram_tensor("ep", [BSH, T], bf16, kind="ExternalInput")
    w_d = nc.dram_tensor("wmat", [P, 2 * P], bf16, kind="ExternalInput")  # [-4I | 2.5I]
    y_d = nc.dram_tensor("y", [BSH, T], bf16, kind="ExternalOutput")

    dtr = dt_d.rearrange("(n p) t -> n p t", p=P)
    epr = ep_d.rearrange("(n p) t -> n p t", p=P)
    yr = y_d.rearrange("(n p) t -> n p t", p=P)

    mult = mybir.AluOpType.mult
    add = mybir.AluOpType.add
    Copy = mybir.ActivationFunctionType.Copy

    def cs(c):
        return slice(c * L, (c + 1) * L)

    from contextlib import ExitStack
    es = ExitStack()
    sb = lambda name, shape, dtype: es.enter_context(nc.sbuf_tensor(name, shape, dtype))
    pt = lambda name, shape: es.enter_context(nc.psum_tensor(name, shape, f32))
    sem = lambda name: es.enter_context(nc.semaphore(name))
    with es:
        dts = [sb(f"dts{s}", [P, T], bf16) for s in range(3)]
        eps = [sb(f"eps{s}", [P, T], bf16) for s in range(3)]
        sm = [sb(f"sm{s}", [P, L], bf16) for s in range(3)]
        g = [sb(f"g{s}", [P, L], bf16) for s in range(3)]
        sg = [sb(f"sg{s}", [P, T], bf16) for s in range(2)]
        ws = sb("ws", [P, 2 * P], bf16)
        pa = [pt(f"pa{s}", [P, L]) for s in range(2)]
        pq = [pt(f"pq{s}", [P, L]) for s in range(2)]
        in_w = sem("in_w")
        a_sem = sem("a_sem")
        m_sem = sem("m_sem")
        scan_sem = sem("scan_sem")
        pe_sem = sem("pe_sem")
        dc_sem = sem("dc_sem")
        block = es.enter_context(nc.Block(no_gpsimd_drain=True))
        st_sem = [[nc.alloc_semaphore(f"st{s}_{c}") for c in range(CH)] for s in range(2)]
        in_sem = [nc.alloc_semaphore(f"in{s}") for s in range(3)]
        in0_sems = [nc.alloc_semaphore(f"in0c{c}") for c in range(CH)]
        wn4 = ws[:, 0:P]       # -4 * I
        w25 = ws[:, P:2 * P]   # 2.5 * I

        # variable-width chunk table: small chunks at the start (pipeline
        # fills sooner) and at the end (shorter drain chain)
        W0 = [L // 2, L // 2, L, L, L]
        WLAST = [L, L, L, L // 2, L // 2]
        CHUNKS = []            # (tile, col0, width, idx_in_tile)
        for i in range(NT):
            widths = W0 if i == 0 else (WLAST if i == NT - 1 else [L] * CH)
            col = 0
            for k, w in enumerate(widths):
                CHUNKS.append((i, col, w, k))
                col += w
            assert col == T
        NQX = len(CHUNKS)
        qstart = {}
        for q, (i, c0, w, k) in enumerate(CHUNKS):
            if k == 0:
                qstart[i] = q

        # store bookkeeping: region r = c0 // L; cumulative store counts per
        # (tile parity, region) through tile j, for the sg WAR waits
        cum_st = {}
        run = {}
        for j in range(NT):
            for (i, c0, w, k) in CHUNKS:
                if i != j:
                    continue
                key = (i % 2, c0 // L)
                run[key] = run.get(key, 0) + 1
            for s in range(2):
                for r in range(CH):
                    cum_st[(s, r, j)] = run.get((s, r), 0)

        def in_ready_chunk(eng, i, c0):
            if i == 0:
                eng.wait_ge(in0_sems[c0 // L], 32)
            else:
                eng.wait_ge(in_sem[i % 3], 32)

        @block.sync
        def _(sync):
            # tile 0: chunk-granular dt loads on the SP ring; the eps chunks
            # ride the ACT ring in parallel so the first compute chunk is
            # gated by 2x256KB of overlapped transfer, not 2MB
            for c in range(CH):
                sync.dma_start(dts[0][:, cs(c)], dtr[0][:, cs(c)]).then_inc(in0_sems[c], 16)
                if c == 0:
                    sync.dma_start(ws[:, :], w_d[:, :]).then_inc(in_w, 16)
            for i in range(1, NT):
                if i >= 3:
                    # slot reuse: consumers of tile i-3 are done
                    sync.wait_ge(a_sem, qstart[i - 2])
                    sync.wait_ge(m_sem, qstart[i - 2])
                    sync.wait_ge(pe_sem, qstart[i - 2])
                sync.dma_start(dts[i % 3][:, :], dtr[i][:, :]).then_inc(in_sem[i % 3], 16)
                sync.dma_start(eps[i % 3][:, :], epr[i][:, :]).then_inc(in_sem[i % 3], 16)
            for q, (i, c0, w, k) in enumerate(CHUNKS):
                sync.wait_ge(dc_sem, q + 1)
                sync.dma_start(yr[i][:, c0:c0 + w], sg[i % 2][:, c0:c0 + w]).then_inc(
                    st_sem[i % 2][c0 // L], 16)
            for s in range(2):
                for r in range(CH):
                    if cum_st[(s, r, NT - 1)]:
                        sync.wait_ge(st_sem[s][r], 16 * cum_st[(s, r, NT - 1)])

        @block.scalar
        def _(scalar):
            # eps tile-0 chunk loads ride the ACT ring (parallel to SP's ring)
            for c in range(CH):
                scalar.dma_start(eps[0][:, cs(c)], epr[0][:, cs(c)]).then_inc(in0_sems[c], 16)
            # warmup: trigger the lazy ACT_TABLE_LOAD while inputs stream in
            scalar.activation(sg[0][:, 0:1], sg[0][:, 0:1], Copy)

            def dc(q):
                i, c0, w, k = CHUNKS[q]
                scalar.wait_ge(pe_sem, q + 1)
                if i >= 2:
                    scalar.wait_ge(st_sem[i % 2][c0 // L],
                                   16 * cum_st[(i % 2, c0 // L, i - 2)])
                scalar.activation(sg[i % 2][:, c0:c0 + w], pq[q % 2][:, 0:w], Copy,
                                  bias=0.0, scale=1.0).then_inc(dc_sem, 1)

            for q, (i, c0, w, k) in enumerate(CHUNKS):
                in_ready_chunk(scalar, i, c0)
                if q >= 2:
                    # pa ring WAR: scan(q-2) has read pa[q%2]
                    scalar.wait_ge(scan_sem, q - 1)
                scalar.activation(pa[q % 2][:, 0:w], dts[i % 3][:, c0:c0 + w], Copy,
                                  bias=1.0, scale=-2.0).then_inc(a_sem, 1)
                if q >= 2:
                    dc(q - 2)
            for q in (NQX - 2, NQX - 1):
                dc(q)

        @block.gpsimd
        def _(gpsimd):
            for q, (i, c0, w, k) in enumerate(CHUNKS):
                in_ready_chunk(gpsimd, i, c0)
                if q >= 3:
                    # sm ring WAR: scan(q-3) has read sm[q%3]
                    gpsimd.wait_ge(scan_sem, q - 2)
                gpsimd.tensor_tensor(
                    sm[q % 3][:, 0:w], dts[i % 3][:, c0:c0 + w],
                    eps[i % 3][:, c0:c0 + w], mult).then_inc(m_sem, 1)

        @block.vector
        def _(vector):
            for q, (i, c0, w, k) in enumerate(CHUNKS):
                vector.wait_ge(a_sem, q + 1)
                vector.wait_ge(m_sem, q + 1)
                if q >= 3:
                    # g ring WAR: PE(q-3) has read g[q%3]
                    vector.wait_ge(pe_sem, q - 2)
                if k > 0:
                    # init reads g[q-1][:, wprev-1]: self-wait until the
                    # previous scan's SBUF write has drained (sem incs fire
                    # post-drain; the engine retires before the write lands)
                    vector.wait_ge(scan_sem, q)
                    wprev = CHUNKS[q - 1][2]
                    init = g[(q - 1) % 3][:, wprev - 1:wprev]
                else:
                    init = 0.0
                vector.tensor_tensor_scan(
                    g[q % 3][:, 0:w], pa[q % 2][:, 0:w], sm[q % 3][:, 0:w],
                    init, mult, add).then_inc(scan_sem, 1)

        @block.tensor
        def _(tensor):
            tensor.wait_ge(in_w, 16)
            for q, (i, c0, w, k) in enumerate(CHUNKS):
                tensor.wait_ge(scan_sem, q + 1)
                if q >= 2:
                    # pq ring WAR: dc(q-2) has drained pq[q%2]
                    tensor.wait_ge(dc_sem, q - 1)
                nh = w // 512
                # -4*g then +2.5*eps per 512-col half; one weight swap per chunk
                for h in range(nh):
                    tensor.matmul(pq[q % 2][:, h * 512:(h + 1) * 512], wn4,
                                  g[q % 3][:, h * 512:(h + 1) * 512],
                                  start=True, stop=False, skip_group_check=True)
                for h in range(nh):
                    mm = tensor.matmul(pq[q % 2][:, h * 512:(h + 1) * 512], w25,
                                       eps[i % 3][:, c0 + h * 512:c0 + (h + 1) * 512],
                                       start=False, stop=True, skip_group_check=True)
                mm.then_inc(pe_sem, 1)

    return nc


_NC_CACHE: dict = {}


def _get_nc() -> bass.Bass:
    if "nc" not in _NC_CACHE:
        _NC_CACHE["nc"] = build_nc()
    return _NC_CACHE["nc"]


def _make_w() -> np.ndarray:
    w = np.zeros((P, 2 * P), np.float32)
    w[:, 0:P] = np.eye(P) * -4.0
    w[:, P:2 * P] = np.eye(P) * 2.5
    return w.astype(ml_dtypes.bfloat16)


def run(x: np.ndarray, trace: bool = False):
    b, t_len, c = x.shape
    assert c == 2 and b == N_CORES * BSH and t_len == T
    x = np.asarray(x, dtype=np.float32)
    ep = np.ascontiguousarray(x[:, :, 0]).astype(ml_dtypes.bfloat16)
    dt = np.ascontiguousarray(x[:, :, 1]).astype(ml_dtypes.bfloat16)
    w = _make_w()
    eps_sh = ep.reshape(N_CORES, BSH, T)
    dts_sh = dt.reshape(N_CORES, BSH, T)
    in_maps = [{"dt": dts_sh[i], "ep": eps_sh[i], "wmat": w} for i in range(N_CORES)]
    res = run_bass_kernel_spmd(
        _get_nc(), in_maps, core_ids=list(range(N_CORES)), trace=trace,
    )
    out = np.concatenate([r["y"].astype(np.float32) for r in res.results], axis=0)
    return out.reshape(b, t_len, 1), res


def kernel(x: np.ndarray) -> np.ndarray:
    out, _ = run(x, trace=False)
    return out
